# revision 39
# baseline (speedup 1.0000x reference)
"""Block-diagonal 2x2 equalizer kernel for Trainium2 (8 NeuronCores), v3.1.

Per point (b, u, s, f) solves the 2x2 system M x = v by Cramer's rule:
    m_ij = h[b, pi[u], i, 0, 2u+j, s, f]   (only 1/4 of h is needed)
    det  = m00*m11 - m01*m10               (fp32: min |det| ~ 1.5e-4, so the
                                            det chain MUST stay fp32 - 16-bit
                                            h makes det cross zero)
    x0   = (m11*v0 - m01*v1) / det         (numerators tolerate fp16)
    x1   = (m00*v1 - m10*v0) / det

Sharding: pure data parallel over batch, 2 batches per core on 8 cores.
I/O per core: h planes fp32 (3.67MB), v planes fp16 (0.46MB), x out fp16
(0.46MB). Host does gather/transpose/dtype-pack only; all arithmetic is
on-device.

Engine split, NCH=8 chunks (FC=224, WC=448):
  DVE:  per chunk: QQ = {Af|Bf}*V_broadcast, one [128,896] fp16 op at the
        HW 2x_1p rate (plain TensorTensor has a 2x uop program; the fused
        TensorScalarPtr measured 1x); P = A*B wide fp32 {p0|p1};
        det = P0-P1. Per chunk PAIR (pairs 0-2): R = {q0|q2}-{q1|q3} and
        X = R*rdet_broadcast as single [128,2,2,FC] strided ops (2x_1p
        holds for 4-dim APs). The last pair runs per-chunk R/X so chunk
        6's X and store overlap the last fp32 slot and the final
        det->recip->R->X->store chain is short.
  ACT (parallel scalar engine): a [128,1] dummy activation first, forcing
        the one-time ACT_TABLE_LOAD (~1.3us) during the first DMA flight;
        then per chunk Af = cvt(A) wide, Bf = swapped-half cvts of B ->
        {m10f|m00f}, Reciprocal spline (fp32 in, fp16 out, 2.2e-5 rel
        err). Runs concurrently with DVE without contention (GPSIMD does
        contend - measured previously - and stays unused).
  SYNC: inputs on one ring, each chunk split into an A+B DMA (fp32,
        3584B rows) and a V DMA (fp16, 896B rows), issue order ab0, ab1,
        v0, ab2, v1, ... so the fp32 parts the det chain waits on are
        front-loaded and the DGE ring's one-time ~1.5us warm-up stall
        lands on deferrable V traffic; then pair stores + 2 half stores,
        final outS wait.

Packing: A = {m11|m01}, B = {m00|m10}, V = {v0|v1} (fp16):
  P  = A*B = {m11*m00 | m01*m10} = {p0|p1}
  Af = {m11f|m01f},  Bf = {m10f|m00f}
  QQ = {Af|Bf}*{V|V} = {q0|q1|q3|q2}   (q0=m11f*v0 q1=m01f*v1
                                        q2=m00f*v1 q3=m10f*v0)
  R  = {q0|q2} - {q1|q3} = {r0|r1} per chunk, X = R*rdet = {x0|x1}

Dataflow rules: every SBUF region written exactly once (no WAR hazards),
all waits standalone wait_ge (walrus single-wait rule), cross-engine waits
get >= 1 chunk of slack so semaphore propagation never stalls a hot pipe.
Block(no_gpsimd_drain=True) skips the slow gpsimd dge_drain at teardown.

Timeline on HW (full clock): ~2.6us to first DMA byte (framework preamble
+ ring start), DVE streams from ~4.4us to ~21.8us (DVE-bound; busy
15.7us, ACT 15.3us concurrently - both elementwise engines saturated),
~10us fixed runtime-emitted NEFF finalization tail (no BIR attribution).
Baseline 43.2us -> ~32.0-32.9us at full clock; the device DVFS-throttles
intermittently (+15-20% on everything), so absolute numbers drift.
"""

from contextlib import ExitStack

import numpy as np

import concourse.bass as bass
import concourse.mybir as mybir
from concourse.bass_utils import run_bass_kernel_spmd

# Problem shapes (hardcoded per contract)
B, U, A, NTX, T, S, F = 16, 4, 2, 1, 8, 14, 2048
SF = S * F               # 28672
NCORES = 8
BPC = B // NCORES        # 2 batches per core
PTS = BPC * U * SF       # 229376 points per core
COLS = PTS // 128        # 1792
NCH = 8                  # pipeline chunks
FC = COLS // NCH         # 224
WC = 2 * FC              # 448
ABROW = 2 * WC * 4       # A+B fp32 bytes per partition row (3584)
VROW = WC * 2            # V fp16 bytes per partition row (896)
ROW = ABROW + VROW       # 5376

TRACE = False
LAST_RESULTS = None

f32 = mybir.dt.float32
f16 = mybir.dt.float16
u8 = mybir.dt.uint8


def _build_nc():
    nc = bass.Bass("TRN2")
    dIn = nc.dram_tensor("dIn", [NCH, 128, ROW], u8, kind="ExternalInput")
    xO = nc.dram_tensor("xO", [NCH // 2, 128, 2 * WC], f16, kind="ExternalOutput")

    with ExitStack() as ctx:
        sb = lambda name, w, dt: ctx.enter_context(nc.sbuf_tensor(name, [128, w], dt))
        tIn = [sb(f"tIn{k}", ROW, u8) for k in range(NCH)]
        tABf = [sb(f"tABf{k}", 2 * WC, f16) for k in range(NCH)]
        tP = [sb(f"tP{k}", WC, f32) for k in range(NCH)]
        tDet = [sb(f"tDet{k}", FC, f32) for k in range(NCH)]
        tRda = sb("tRda", NCH * FC, f16)
        tWarm = sb("tWarm", 1, f16)
        tQa = sb("tQa", NCH * 2 * WC, f16)
        tRa = sb("tRa", NCH * WC, f16)
        tX = sb("tX", NCH * WC, f16)

        vA = [tIn[k][:, 0:WC * 4].bitcast(f32) for k in range(NCH)]
        vB = [tIn[k][:, WC * 4:ABROW].bitcast(f32) for k in range(NCH)]
        vV = [tIn[k][:, ABROW:ROW].bitcast(f16) for k in range(NCH)]

        inSem = [ctx.enter_context(nc.semaphore(f"inS{k}")) for k in range(NCH)]
        vSem = [ctx.enter_context(nc.semaphore(f"inV{k}")) for k in range(NCH)]
        dveS = ctx.enter_context(nc.semaphore("dveS"))
        actS = ctx.enter_context(nc.semaphore("actS"))
        outS = ctx.enter_context(nc.semaphore("outS"))

        NP = NCH // 2
        det_idx = [0] * NCH
        x_idx = [0] * (NP + 1)
        cvt_idx = [0] * NCH
        recip_idx = [0] * NCH
        # DVE slot t: QQ(t-1), fp32(t), then joint pair p=(t-2)//2 on even t
        # (pairs 0..NP-2); the last pair runs per-chunk R/X at slots NCH-1
        # and NCH so its first half overlaps the last fp32 and its store
        # chain is short
        dc = 0
        for t in range(NCH + 1):
            if 1 <= t <= NCH:
                dc += 1
            if t < NCH:
                dc += 2
                det_idx[t] = dc
            if t >= 2 and t % 2 == 0 and (t - 2) // 2 < NP - 1:
                dc += 2
                x_idx[(t - 2) // 2] = dc
            if t >= NCH - 1:
                dc += 2
                x_idx[NP - 1 + (t - (NCH - 1))] = dc
        # ACT slot t: recip(t-1) first, then converts(t)
        ac = 0
        for t in range(NCH + 1):
            if 1 <= t <= NCH:
                ac += 1
                recip_idx[t - 1] = ac
            if t < NCH:
                ac += 3
                cvt_idx[t] = ac

        with nc.Block(no_gpsimd_drain=True) as block:

            @block.scalar
            def _(scalar):
                # dummy activation with no data dependency: forces the one-time
                # ACT_TABLE_LOAD (~1.3us) to run during the first DMA flight
                # instead of after chunk 0 lands
                scalar.copy(tWarm[:], nc.const_aps.aps[(f32, 0.0)])
                for t in range(NCH + 1):
                    if 1 <= t <= NCH:
                        k = t - 1
                        scalar.wait_ge(dveS, det_idx[k])
                        scalar.add_instruction(
                            mybir.InstActivation(
                                name=nc.get_next_instruction_name(),
                                func=mybir.ActivationFunctionType.Reciprocal,
                                ins=[
                                    scalar.lower_ap(tDet[k][:]),
                                    mybir.ImmediateValue(dtype=f32, value=0.0),
                                    mybir.ImmediateValue(dtype=f32, value=1.0),
                                    mybir.ImmediateValue(dtype=f32, value=0.0),
                                ],
                                outs=[scalar.lower_ap(tRda[:, k * FC:(k + 1) * FC])],
                            )
                        ).then_inc(actS, 1)
                    if t < NCH:
                        scalar.wait_ge(inSem[t], 16)
                        scalar.copy(tABf[t][:, :WC], vA[t]).then_inc(actS, 1)
                        scalar.copy(tABf[t][:, WC:WC + FC], vB[t][:, FC:]).then_inc(
                            actS, 1
                        )
                        scalar.copy(tABf[t][:, WC + FC:], vB[t][:, :FC]).then_inc(
                            actS, 1
                        )

            @block.sync
            def _(sync):
                # ab(k) carries the fp32 A+B planes, v(k) the fp16 V plane.
                # Issue order ab0, ab1, v0, ab2, v1, ... front-loads the fp32
                # parts the det chain is waiting on.
                def _ab(k):
                    sync.dma_start(
                        out=tIn[k][:, :ABROW], in_=dIn[k, :, :ABROW]
                    ).then_inc(inSem[k], 16)

                def _v(k):
                    sync.dma_start(
                        out=tIn[k][:, ABROW:], in_=dIn[k, :, ABROW:]
                    ).then_inc(vSem[k], 16)

                _ab(0)
                _ab(1)
                _v(0)
                for k in range(2, NCH):
                    _ab(k)
                    _v(k - 1)
                _v(NCH - 1)
                for p in range(NP - 1):
                    sync.wait_ge(dveS, x_idx[p])
                    sync.dma_start(
                        out=xO[p], in_=tX[:, p * 2 * WC:(p + 1) * 2 * WC]
                    ).then_inc(outS, 16)
                for i, k in enumerate((NCH - 2, NCH - 1)):
                    sync.wait_ge(dveS, x_idx[NP - 1 + i])
                    sync.dma_start(
                        out=xO[NP - 1, :, i * WC:(i + 1) * WC],
                        in_=tX[:, k * WC:(k + 1) * WC],
                    ).then_inc(outS, 16)
                sync.wait_ge(outS, (NP + 1) * 16)

            @block.vector
            def _(vector):
                for t in range(NCH + 1):
                    if 1 <= t <= NCH:
                        k = t - 1
                        vector.wait_ge(actS, cvt_idx[k])
                        vector.wait_ge(vSem[k], 16)
                        qk = tQa[:, k * 2 * WC:(k + 1) * 2 * WC]
                        qq = qk.rearrange("p (a c) -> p a c", a=2, c=WC)
                        vbc = vV[k].unsqueeze(1).broadcast_to((128, 2, WC))
                        abf = tABf[k][:].rearrange("p (a c) -> p a c", a=2, c=WC)
                        vector.tensor_mul(qq, abf, vbc).then_inc(dveS, 1)
                    if t < NCH:
                        vector.wait_ge(inSem[t], 16)
                        vector.tensor_mul(tP[t][:], vA[t], vB[t]).then_inc(dveS, 1)
                        vector.tensor_sub(
                            tDet[t][:], tP[t][:, :FC], tP[t][:, FC:]
                        ).then_inc(dveS, 1)
                    if t >= NCH - 1:
                        k = NCH - 2 if t == NCH - 1 else NCH - 1
                        vector.wait_ge(actS, recip_idx[k])
                        q4 = tQa[:, k * 2 * WC:(k + 1) * 2 * WC].rearrange(
                            "p (a c) -> p a c", a=4, c=FC
                        )
                        rr1 = tRa[:, k * WC:(k + 1) * WC].rearrange(
                            "p (a c) -> p a c", a=2, c=FC
                        )
                        vector.tensor_sub(rr1, q4[:, 0::3], q4[:, 1:3]).then_inc(
                            dveS, 1
                        )
                        xx1 = tX[:, k * WC:(k + 1) * WC].rearrange(
                            "p (a c) -> p a c", a=2, c=FC
                        )
                        rdb1 = (
                            tRda[:, k * FC:(k + 1) * FC]
                            .unsqueeze(1)
                            .broadcast_to((128, 2, FC))
                        )
                        vector.tensor_mul(xx1, rr1, rdb1).then_inc(dveS, 1)
                    if t >= 2 and t % 2 == 0 and (t - 2) // 2 < NP - 1:
                        p = (t - 2) // 2
                        vector.wait_ge(actS, recip_idx[2 * p + 1])
                        q8 = tQa[:, p * 4 * WC:(p + 1) * 4 * WC].rearrange(
                            "p (a b c) -> p a b c", a=2, b=4, c=FC
                        )
                        rr = tRa[:, p * 2 * WC:(p + 1) * 2 * WC].rearrange(
                            "p (a b c) -> p a b c", a=2, b=2, c=FC
                        )
                        vector.tensor_sub(
                            rr, q8[:, :, 0::3], q8[:, :, 1:3]
                        ).then_inc(dveS, 1)
                        xx = tX[:, p * 2 * WC:(p + 1) * 2 * WC].rearrange(
                            "p (a b c) -> p a b c", a=2, b=2, c=FC
                        )
                        rdb = (
                            tRda[:, p * 2 * FC:(p + 1) * 2 * FC]
                            .rearrange("p (a c) -> p a c", a=2, c=FC)
                            .unsqueeze(2)
                            .broadcast_to((128, 2, 2, FC))
                        )
                        vector.tensor_mul(xx, rr, rdb).then_inc(dveS, 1)

    return nc


def _chunk(plane):
    """[128*COLS] flat (C-order over [BPC,U,S,F]) -> [NCH, 128, FC]."""
    return plane.reshape(128, NCH, FC).transpose(1, 0, 2)


def make_in_maps(y, h, precoding_ind):
    """Host-side gather + byte-pack. Returns per-core input maps."""
    y = np.asarray(y)
    h = np.asarray(h)
    pi = np.asarray(precoding_ind).astype(np.int64)

    hg = h[:, pi[0]]                                     # [B, U, A, NTX, T, S, F]
    hsel = np.stack(
        [hg[:, u, :, 0, 2 * u:2 * u + 2] for u in range(U)], axis=1
    )                                                    # [B, U, A(i), 2(j), S, F]
    hsel = np.ascontiguousarray(hsel).astype(np.float32)
    yr = np.ascontiguousarray(y).astype(np.float32)      # [B, U, A, S, F]

    in_maps = []
    for c in range(NCORES):
        b0 = c * BPC
        hs = hsel[b0:b0 + BPC]
        ys = yr[b0:b0 + BPC]
        m00 = np.ascontiguousarray(hs[:, :, 0, 0]).reshape(-1)
        m01 = np.ascontiguousarray(hs[:, :, 0, 1]).reshape(-1)
        m10 = np.ascontiguousarray(hs[:, :, 1, 0]).reshape(-1)
        m11 = np.ascontiguousarray(hs[:, :, 1, 1]).reshape(-1)
        v0 = np.ascontiguousarray(ys[:, :, 0]).reshape(-1)
        v1 = np.ascontiguousarray(ys[:, :, 1]).reshape(-1)
        hA = np.concatenate([_chunk(m11), _chunk(m01)], axis=2)  # [NCH,128,WC] f32
        hB = np.concatenate([_chunk(m00), _chunk(m10)], axis=2)
        yV = np.concatenate(
            [_chunk(v0).astype(np.float16), _chunk(v1).astype(np.float16)], axis=2
        )                                                        # {v0|v1}
        dIn = np.concatenate(
            [
                hA.view(np.uint8).reshape(NCH, 128, WC * 4),
                hB.view(np.uint8).reshape(NCH, 128, WC * 4),
                yV.view(np.uint8).reshape(NCH, 128, VROW),
            ],
            axis=2,
        )                                                        # [NCH,128,ROW]
        in_maps.append({"dIn": np.ascontiguousarray(dIn)})
    return in_maps


def _unchunk(t):
    """[NCH, 128, FC] -> [128*COLS] flat."""
    return t.transpose(1, 0, 2).reshape(-1)


def assemble_output(results):
    """Per-core xO [NCH, 128, WC] f16 -> full [B, U, A, S, F] f32."""
    out = np.empty((B, U, A, S, F), np.float32)
    for c in range(NCORES):
        xo = np.asarray(results[c]["xO"]).astype(np.float32)
        xo = xo.reshape(NCH // 2, 128, 2, WC).transpose(0, 2, 1, 3).reshape(
            NCH, 128, WC
        )
        x0 = _unchunk(xo[:, :, :FC]).reshape(BPC, U, S, F)
        x1 = _unchunk(xo[:, :, FC:]).reshape(BPC, U, S, F)
        out[c * BPC:(c + 1) * BPC, :, 0] = x0
        out[c * BPC:(c + 1) * BPC, :, 1] = x1
    return out


def kernel(y, h, precoding_ind):
    global LAST_RESULTS
    in_maps = make_in_maps(y, h, precoding_ind)
    nc = _build_nc()
    res = run_bass_kernel_spmd(nc, in_maps, list(range(NCORES)), trace=TRACE)
    LAST_RESULTS = res
    return assemble_output(res.results)
